# revision 33
# baseline (speedup 1.0000x reference)
"""Trainium2 Bass kernel: multi-relation GNN message-passing layer (H2FDMultiRelationLayer).

Math folds (exact):
  * sign(tanh(x)) == sign(x); concat([sd,dd,sd-dd]) @ fW == sd@(f1+f3) + dd@(f2-f3)
    with per-node scalars u = h@du + cu, v = h@dv + cv so sign_e = sign(u[src]+v[dst]).
    u, v are relation-independent (dW shared), computed once in fp32.
  * attention logit per head a: alpha[e,a] = leaky_relu(sign_e*p[src,a] + q[dst,a])
    with p = h@(wW@P)+bias, q = h@(wW@Q)+bias+ab  (per-node, per-head scalars).
  * segment softmax without max-subtraction (alpha bounded; exp cannot overflow).

v3 performance structure:
  * All heavy matmuls bf16; fp32 kept only for the [u,v,q*12] node-scalar matmul.
  * Node tables: bigT [R, n_pad, 384] bf16 rows [hw(256)|p(4)|u_hi|u_lo|pad]
    (768B rows — dma_gather needs 256B-aligned rows); sdT [n_pad, 16] fp32.
  * Phase-2 src gathers via InstDMAGatherAnt: ONE instruction per (block,
    relation, node-half) gathers all ~K*128 edge rows (994ns fixed amortized).
    int16 index limit forces the two node-half tables; host splits edge slots
    into the A-half columns [0,KA) and B-half [KA,K2).
  * dst-side per-edge values (v_hi,v_lo,q) expanded on the PE via one-hot mofT
    matmuls from an indirectly-fetched local 128-row slice — no dst gathers.
  * mof/mofT one-hot matrices for ALL K slots built in ONE wide DVE op each;
    pad slots carry dst-offset 128 so their one-hot column is all-zero — pads
    are killed by the scatter matmul itself, no edge mask anywhere.
  * leaky-relu+exp on the Scalar engine; phase-1 bias adds split across
    Vector/GpSimd; biases via partition-broadcast DMA tiles (no bias matmuls).
  * Phase-1 h loads and sdT writes batched 4 tiles per DMA; bigT written once
    per tile for all 3 relations via a transposed DRAM access pattern.

Distribution: nodes partitioned by destination across 8 cores; each core gets
exactly the edges whose dst it owns (host-side selection, dst-sorted, grouped
into 128-row destination blocks). Node tables computed replicated. No collectives.
"""

import math
from contextlib import ExitStack

import numpy as np
import ml_dtypes

import concourse.bass as bass
import concourse.bacc as bacc
import concourse.tile as tile
import concourse.mybir as mybir
from concourse.bass_utils import run_bass_kernel_spmd
from concourse.masks import make_identity

# problem dims (fixed by the nn.Module)
IN = 128          # input feature dim
HF = 64           # per-head hidden
AH = 4            # attention heads
R = 3             # relations
H = AH * HF       # 256
NCORES = 8
P = 128
BW = 260          # per-relation bf16 matmul cols: [hw(256) | p(4)]
TW = 384          # bf16 table row: [hw(256)|p(4)|u_hi|u_lo|pad] — 768B, 256B-aligned
CW = 260          # scatter rhs cols: [hwv(256) | exp(4)]
F32 = mybir.dt.float32
BF16 = mybir.dt.bfloat16
I32 = mybir.dt.int32
I16 = mybir.dt.int16

_PROG_CACHE: dict = {}


def _build_program(nt: int, nblocks: int, ka: int, kb: int, ncores: int):
    """Trace + compile the SPMD device program (same for all cores)."""
    n_pad = nt * P            # padded node-table rows
    nh = n_pad // 2           # node-half split for int16 gather indices
    k2 = ka + kb              # edge-group slots per block
    bk = nblocks * k2
    npcp = nblocks * P        # padded per-core output rows

    nc = bacc.Bacc("TRN2", target_bir_lowering=False, debug=False, num_devices=ncores)

    hT = nc.dram_tensor("hT", [IN, n_pad], F32, kind="ExternalInput")
    bigW = nc.dram_tensor("bigW", [R, IN, BW], BF16, kind="ExternalInput")
    bigB = nc.dram_tensor("bigB", [R, 1, BW], F32, kind="ExternalInput")
    sdW = nc.dram_tensor("sdW", [IN, 16], F32, kind="ExternalInput")
    sdB = nc.dram_tensor("sdB", [1, 16], F32, kind="ExternalInput")
    linW6 = nc.dram_tensor("linW6", [2 * R, P, H], BF16, kind="ExternalInput")
    linB = nc.dram_tensor("linB", [1, H], F32, kind="ExternalInput")
    sidx16 = nc.dram_tensor("sidx16", [R, P, bk * 8], I16, kind="ExternalInput")
    gcnt = nc.dram_tensor("gcnt", [R, 1, nblocks * 2], I32, kind="ExternalInput")
    offs = nc.dram_tensor("offs", [R, P, bk], BF16, kind="ExternalInput")
    offsT = nc.dram_tensor("offsT", [R, bk, P], BF16, kind="ExternalInput")
    cbase = nc.dram_tensor("cbase", [1, 1], I32, kind="ExternalInput")
    out = nc.dram_tensor("out", [npcp, H], F32, kind="ExternalOutput")

    bigT = nc.dram_tensor("bigT", [R, n_pad, TW], BF16)
    sdT = nc.dram_tensor("sdT", [n_pad, 16], F32)

    with tile.TileContext(nc) as tc:
        with ExitStack() as ctx:
            cpool = ctx.enter_context(tc.tile_pool(name="const", bufs=1))
            iota_i = cpool.tile([P, P], I32)
            nc.gpsimd.iota(iota_i[:], pattern=[[1, P]], base=0, channel_multiplier=0)
            iota_b = cpool.tile([P, P], BF16)
            nc.vector.tensor_copy(iota_b[:], iota_i[:])
            iotac_i = cpool.tile([P, 1], I32)
            nc.gpsimd.iota(iotac_i[:], pattern=[[0, 1]], base=0, channel_multiplier=1)
            iotac_f = cpool.tile([P, 1], F32)
            nc.vector.tensor_copy(iotac_f[:], iotac_i[:])
            ident = cpool.tile([P, P], BF16)
            make_identity(nc, ident[:])

            bw_sb = []
            for r in range(R):
                t = cpool.tile([IN, BW], BF16, tag=f"bw{r}")
                nc.sync.dma_start(t[:], bigW[r, :, :])
                bw_sb.append(t)
            bb_sb = []
            for r in range(R):
                t = cpool.tile([P, BW], F32, tag=f"bb{r}")
                nc.sync.dma_start(t[:], bigB[r, :, :].to_broadcast((P, BW)))
                bb_sb.append(t)
            sdw_sb = cpool.tile([IN, 16], F32)
            nc.sync.dma_start(sdw_sb[:], sdW[:, :])
            sdb_sb = cpool.tile([P, 16], F32)
            nc.sync.dma_start(sdb_sb[:], sdB[:, :].to_broadcast((P, 16)))
            lw_sb = []
            for i in range(2 * R):
                t = cpool.tile([P, H], BF16, tag=f"lw{i}")
                nc.sync.dma_start(t[:], linW6[i, :, :])
                lw_sb.append(t)
            lb_sb = cpool.tile([P, H], F32)
            nc.sync.dma_start(lb_sb[:], linB[:, :].to_broadcast((P, H)))
            cb_sb = cpool.tile([P, 1], I32)
            nc.sync.dma_start(cb_sb[:], cbase[:, :].to_broadcast((P, 1)))
            cb_f = cpool.tile([P, 1], F32)
            nc.vector.tensor_copy(cb_f[:], cb_sb[:])
            si_sb, of_sb, gc_sb = [], [], []
            for r in range(R):
                a = cpool.tile([P, bk * 8], I16, tag=f"si{r}")
                nc.sync.dma_start(a[:], sidx16[r, :, :])
                si_sb.append(a)
                a = cpool.tile([P, bk], BF16, tag=f"of{r}")
                nc.sync.dma_start(a[:], offs[r, :, :])
                of_sb.append(a)
                a = cpool.tile([1, nblocks * 2], I32, tag=f"gc{r}")
                nc.sync.dma_start(a[:], gcnt[r, :, :])
                gc_sb.append(a)

            # ---------------- phase 1: node tables ----------------
            GH = 4  # h-load / sdT-write batching
            with tc.tile_pool(name="p1h", bufs=3) as hp, \
                 tc.tile_pool(name="p1ps", bufs=5, space="PSUM") as pp, \
                 tc.tile_pool(name="p1sd", bufs=2, space="PSUM") as sp, \
                 tc.tile_pool(name="p1o", bufs=4) as op, \
                 tc.tile_pool(name="p1s", bufs=2) as sdp:
                for tg in range(0, nt, GH):
                    g = min(GH, nt - tg)
                    ht4 = hp.tile([IN, GH * P], F32, tag="ht4")
                    nc.sync.dma_start(ht4[:, 0:g * P], hT[:, tg * P:(tg + g) * P])
                    htb4 = hp.tile([IN, GH * P], BF16, tag="htb4")
                    nc.gpsimd.tensor_copy(htb4[:, 0:g * P], ht4[:, 0:g * P])
                    sd4 = sdp.tile([P, GH * 16], F32)
                    for j in range(g):
                        t = tg + j
                        ht = ht4[:, j * P:(j + 1) * P]
                        htb = htb4[:, j * P:(j + 1) * P]

                        # fp32 node scalars [u, v, q*12]
                        ps_sd = sp.tile([P, 16], F32)
                        nc.tensor.matmul(ps_sd[:], lhsT=ht, rhs=sdw_sb[:],
                                         start=True, stop=True)
                        sd = sd4[:, j * 16:(j + 1) * 16]
                        nc.vector.tensor_add(sd, ps_sd[:], sdb_sb[:])

                        # u -> bf16 hi/lo (shared across relations)
                        uhl = op.tile([P, 2], BF16, tag="uhl")
                        nc.gpsimd.tensor_copy(uhl[:, 0:1], sd[:, 0:1])
                        uhf = op.tile([P, 1], F32, tag="uhf")
                        nc.gpsimd.tensor_copy(uhf[:], uhl[:, 0:1])
                        nc.gpsimd.tensor_sub(uhl[:, 1:2], sd[:, 0:1], uhf[:])

                        hw3 = op.tile([P, R * TW], BF16, tag="hw3")
                        hw3v = hw3[:].rearrange("p (r c) -> p r c", r=R)
                        for r in range(R):
                            ps = pp.tile([P, BW], F32)
                            nc.tensor.matmul(ps[:], lhsT=htb, rhs=bw_sb[r][:],
                                             start=True, stop=True)
                            nc.vector.tensor_add(hw3[:, r * TW:r * TW + BW],
                                                 ps[:], bb_sb[r][:])
                        nc.gpsimd.tensor_copy(
                            hw3v[:, :, BW:BW + 2],
                            uhl[:].rearrange("p (o c) -> p o c", o=1)
                            .to_broadcast((P, R, 2)))
                        nc.sync.dma_start(
                            bigT[:, t * P:(t + 1) * P, :].rearrange(
                                "r n c -> n r c"),
                            hw3v)
                    nc.sync.dma_start(
                        sdT[tg * P:(tg + g) * P, :].rearrange(
                            "(j n) c -> n j c", j=g),
                        sd4[:, 0:g * 16].rearrange("p (j c) -> p j c", j=g))

            # ---------------- phase 2: edge aggregation ----------------
            with tc.tile_pool(name="gth", bufs=4) as gp, \
                 tc.tile_pool(name="cc", bufs=3) as cp, \
                 tc.tile_pool(name="mo", bufs=3) as mp, \
                 tc.tile_pool(name="sml", bufs=4) as smlp, \
                 tc.tile_pool(name="dv", bufs=3) as dvp, \
                 tc.tile_pool(name="pex", bufs=2, space="PSUM") as pexp, \
                 tc.tile_pool(name="pblk", bufs=2, space="PSUM") as pblkp, \
                 tc.tile_pool(name="pout", bufs=2, space="PSUM") as poutp, \
                 tc.tile_pool(name="ptp", bufs=2, space="PSUM") as ptpp, \
                 tc.tile_pool(name="nrm", bufs=4) as nrmp, \
                 tc.tile_pool(name="obp", bufs=3) as obp:
                # prime the gather buffers: slots past the per-block dynamic
                # gather count are never written, so scrub any NaN bit
                # patterns that fresh SBUF might hold (downstream math must
                # stay finite; pad slots contribute 0 via all-zero one-hots).
                for _ in range(4):
                    t = gp.tile([P, k2 * TW], BF16, tag="gth")
                    nc.vector.memset(t[:], 0.0)
                for b in range(nblocks):
                    pso = poutp.tile([P, H], F32)
                    # local dst-block node scalars (core-dependent rows)
                    lidx_f = dvp.tile([P, 1], F32, tag="lidxf")
                    nc.vector.tensor_scalar(
                        out=lidx_f[:], in0=iotac_f[:],
                        scalar1=cb_f[:], scalar2=float(b * P),
                        op0=mybir.AluOpType.add, op1=mybir.AluOpType.add)
                    lidx = dvp.tile([P, 1], I32, tag="lidx")
                    nc.vector.tensor_copy(lidx[:], lidx_f[:])
                    sdl = dvp.tile([P, 16], F32, tag="sdl")
                    nc.gpsimd.indirect_dma_start(
                        out=sdl[:], out_offset=None,
                        in_=sdT[:, :],
                        in_offset=bass.IndirectOffsetOnAxis(
                            ap=lidx[:, 0:1], axis=0),
                        element_offset=0)
                    # v -> bf16 hi/lo (shared across relations)
                    vhl = dvp.tile([P, 2], BF16, tag="vhl")
                    nc.vector.tensor_copy(vhl[:, 0:1], sdl[:, 1:2])
                    vhf = dvp.tile([P, 1], F32, tag="vhf")
                    nc.vector.tensor_copy(vhf[:], vhl[:, 0:1])
                    nc.vector.tensor_sub(vhl[:, 1:2], sdl[:, 1:2], vhf[:])

                    for r in range(R):
                        csl = slice(b * k2, (b + 1) * k2)
                        dstv = dvp.tile([P, 8], BF16, tag="dstv")
                        nc.scalar.activation(dstv[:, 0:2], vhl[:],
                                             mybir.ActivationFunctionType.Copy)
                        nc.scalar.activation(dstv[:, 2:6],
                                             sdl[:, 2 + 4 * r:6 + 4 * r],
                                             mybir.ActivationFunctionType.Copy)

                        # src gathers: one InstDMAGatherAnt per node-half
                        gth = gp.tile([P, k2 * TW], BF16, tag="gth")
                        nc.gpsimd.dma_gather(
                            out_ap=gth[:, 0:ka * TW].rearrange(
                                "p (k e) -> p k e", k=ka),
                            in_ap=bigT[r, 0:nh, :],
                            idxs_ap=si_sb[r][:, b * k2 * 8:(b * k2 + ka) * 8],
                            num_idxs=ka * P, num_idxs_reg=ka * P,
                            elem_size=TW)
                        nc.gpsimd.dma_gather(
                            out_ap=gth[:, ka * TW:].rearrange(
                                "p (k e) -> p k e", k=kb),
                            in_ap=bigT[r, nh:n_pad, :],
                            idxs_ap=si_sb[r][:, (b * k2 + ka) * 8:
                                             (b + 1) * k2 * 8],
                            num_idxs=kb * P, num_idxs_reg=kb * P,
                            elem_size=TW)

                        # one-hot matrices for ALL k2 slots, one wide op each;
                        # pad slots have of==128 -> all-zero one-hot column.
                        otbs = mp.tile([P, k2 * P], BF16, tag="otbs")
                        nc.sync.dma_start(
                            otbs[:].rearrange("p (k j) -> p k j", k=k2),
                            offsT[r:r + 1, b * k2:(b + 1) * k2, :]
                            .to_broadcast((P, k2, P)))
                        mofT = mp.tile([P, k2 * P], BF16, tag="mofT")
                        nc.vector.tensor_scalar(
                            out=mofT[:], in0=otbs[:],
                            scalar1=iotac_f[:], scalar2=None,
                            op0=mybir.AluOpType.is_equal)
                        mof = mp.tile([P, k2 * P], BF16, tag="mof")
                        nc.vector.tensor_tensor(
                            out=mof[:].rearrange("p (k j) -> p k j", k=k2),
                            in0=iota_b[:].rearrange("p (o j) -> p o j", o=1)
                            .to_broadcast((P, k2, P)),
                            in1=of_sb[r][:, csl].rearrange("p (k o) -> p k o", o=1)
                            .to_broadcast((P, k2, P)),
                            op=mybir.AluOpType.is_equal)

                        # dst-side expand: pex[p, k*8+c] = dstv[of[p,k], c]
                        pex = pexp.tile([P, k2 * 8], F32)
                        for k in range(k2):
                            nc.tensor.matmul(pex[:, k * 8:(k + 1) * 8],
                                             lhsT=mofT[:, k * P:(k + 1) * P],
                                             rhs=dstv[:],
                                             start=True, stop=True)

                        gv = gth[:].rearrange("p (k e) -> p k e", k=k2)
                        ev = pex[:].rearrange("p (k e) -> p k e", k=k2)
                        # score = (u_hi+u_lo) + (v_hi+v_lo); sign
                        scu = smlp.tile([P, k2], F32, tag="scu")
                        scu3 = scu[:].rearrange("p (k o) -> p k o", o=1)
                        nc.vector.tensor_add(scu3, gv[:, :, BW:BW + 1],
                                             gv[:, :, BW + 1:BW + 2])
                        sc = smlp.tile([P, k2], F32, tag="sc")
                        sc3 = sc[:].rearrange("p (k o) -> p k o", o=1)
                        nc.vector.tensor_add(sc3, scu3, ev[:, :, 0:1])
                        nc.vector.tensor_add(sc3, sc3, ev[:, :, 1:2])
                        sgn = smlp.tile([P, k2], BF16, tag="sgn")
                        nc.scalar.sign(sgn[:], sc[:])
                        sgnb = sgn[:].rearrange("p (k o) -> p k o", o=1) \
                            .to_broadcast([P, k2, AH])

                        # alpha = sign*p + q ; leaky relu (ACT); exp (ACT)
                        spq = smlp.tile([P, k2 * AH], F32, tag="spq")
                        spq3 = spq[:].rearrange("p (k a) -> p k a", k=k2)
                        nc.vector.tensor_tensor(out=spq3, in0=gv[:, :, 256:260],
                                                in1=sgnb, op=mybir.AluOpType.mult)
                        nc.vector.tensor_tensor(out=spq3, in0=spq3,
                                                in1=ev[:, :, 2:6],
                                                op=mybir.AluOpType.add)
                        zl = smlp.tile([P, k2 * AH], F32, tag="zl")
                        nc.vector.tensor_scalar_mul(zl[:], spq[:], 0.01)
                        nc.vector.tensor_max(zl[:], zl[:], spq[:])
                        C = cp.tile([P, k2 * CW], BF16)
                        Cv = C[:].rearrange("p (k c) -> p k c", c=CW)
                        exv = Cv[:, :, 256:260]
                        nc.scalar.activation(exv, zl[:].rearrange(
                            "p (k a) -> p k a", k=k2),
                            mybir.ActivationFunctionType.Exp)
                        co = smlp.tile([P, k2 * AH], BF16, tag="co")
                        co3 = co[:].rearrange("p (k a) -> p k a", k=k2)
                        nc.vector.tensor_tensor(out=co3, in0=exv, in1=sgnb,
                                                op=mybir.AluOpType.mult)
                        hwv = Cv[:, :, 0:256].rearrange(
                            "p k (a f) -> p k a f", a=AH)
                        cob = co[:].rearrange("p (k a) -> p k a", k=k2) \
                            .to_broadcast([P, k2, AH, HF])
                        nc.vector.tensor_tensor(
                            out=hwv,
                            in0=gv[:, :, 0:256].rearrange(
                                "p k (a f) -> p k a f", a=AH),
                            in1=cob, op=mybir.AluOpType.mult)

                        # scatter: psb[j, :] = sum_p mof[p,j] * C[p, :]
                        psb = pblkp.tile([P, CW], F32)
                        for k in range(k2):
                            nc.tensor.matmul(psb[:],
                                             lhsT=mof[:, k * P:(k + 1) * P],
                                             rhs=C[:, k * CW:(k + 1) * CW],
                                             start=(k == 0), stop=(k == k2 - 1))

                        den = smlp.tile([P, AH], F32, tag="den")
                        nc.vector.tensor_scalar_max(den[:], psb[:, 256:260], 1e-30)
                        rec = smlp.tile([P, AH], F32, tag="rec")
                        nc.vector.reciprocal(rec[:], den[:])
                        nrm = nrmp.tile([P, H], BF16, tag="nrm")
                        nrm3 = nrm[:].rearrange("p (a f) -> p a f", a=AH)
                        nc.vector.tensor_tensor(
                            out=nrm3,
                            in0=psb[:, 0:256].rearrange("p (a f) -> p a f", a=AH),
                            in1=rec[:].to_broadcast([P, AH, HF]),
                            op=mybir.AluOpType.mult)
                        for ch in range(2):
                            pt = ptpp.tile([P, P], BF16)
                            nc.tensor.transpose(pt[:], nrm[:, ch * P:(ch + 1) * P],
                                                ident[:])
                            ntc = nrmp.tile([P, P], BF16, tag="ntc")
                            nc.scalar.activation(
                                ntc[:], pt[:],
                                mybir.ActivationFunctionType.Copy)
                            nc.tensor.matmul(pso[:], lhsT=ntc[:],
                                             rhs=lw_sb[2 * r + ch][:],
                                             start=(r == 0 and ch == 0),
                                             stop=(r == R - 1 and ch == 1))
                    ob = obp.tile([P, H], F32)
                    nc.vector.tensor_add(ob[:], pso[:], lb_sb[:])
                    nc.sync.dma_start(out[b * P:(b + 1) * P, :], ob[:])

    nc.compile()
    return nc


def _host_prep(h, dW, db, fW, fb, wW, wb, aW, ab, linW, linb, src, dst, ncores):
    """Fold weights + partition/sort edges by destination owner."""
    n = h.shape[0]
    npc = n // ncores
    assert npc * ncores == n
    nblocks = math.ceil(npc / P)
    nt = math.ceil(n / P)
    n_pad = nt * P
    nh = n_pad // 2

    hT = np.zeros((IN, n_pad), np.float32)
    hT[:, :n] = np.ascontiguousarray(h.T)

    fW1, fW2, fW3 = fW[0:H, 0], fW[H:2 * H, 0], fW[2 * H:3 * H, 0]
    du = dW @ (fW1 + fW3)
    dv = dW @ (fW2 - fW3)
    cu = float(db @ (fW1 + fW3) + fb[0])
    cv = float(db @ (fW2 - fW3))

    sdWh = np.zeros((IN, 16), np.float32)
    sdBh = np.zeros((1, 16), np.float32)
    sdWh[:, 0] = du
    sdWh[:, 1] = dv
    sdBh[0, 0] = cu
    sdBh[0, 1] = cv
    bigWh = np.zeros((R, IN, BW), np.float32)
    bigBh = np.zeros((R, 1, BW), np.float32)
    for r in range(R):
        Pm = np.zeros((H, AH), np.float32)
        Qm = np.zeros((H, AH), np.float32)
        for a in range(AH):
            Pm[a * HF:(a + 1) * HF, a] = aW[r, :HF, 0]
            Qm[a * HF:(a + 1) * HF, a] = aW[r, HF:, 0]
        bigWh[r, :, 0:H] = wW[r]
        bigWh[r, :, 256:260] = wW[r] @ Pm
        bigBh[r, 0, 0:H] = wb[r]
        bigBh[r, 0, 256:260] = wb[r] @ Pm
        sdWh[:, 2 + 4 * r:6 + 4 * r] = wW[r] @ Qm
        sdBh[0, 2 + 4 * r:6 + 4 * r] = wb[r] @ Qm + ab[r, 0]

    linW6 = np.ascontiguousarray(
        linW.reshape(2 * R, P, H)).astype(ml_dtypes.bfloat16)
    linB = linb.reshape(1, H).astype(np.float32)

    # --- edge partition: per (relation, core) sort by dst, split by src half ---
    owner = [dst[r] // npc for r in range(R)]
    per_rm = {}
    ka = kb = 1
    for r in range(R):
        for m in range(ncores):
            sel = np.nonzero(owner[r] == m)[0]
            dl = dst[r][sel] - m * npc
            order = np.argsort(dl, kind="stable")
            sel = sel[order]
            dl = dl[order]
            blk = dl // P
            sA = src[r][sel] < nh
            cA = np.bincount(blk[sA], minlength=nblocks)
            cB = np.bincount(blk[~sA], minlength=nblocks)
            ka = max(ka, int(math.ceil(cA.max() / P)))
            kb = max(kb, int(math.ceil(cB.max() / P)))
            per_rm[(r, m)] = (sel, dl, blk, sA)

    k2 = ka + kb
    bk = nblocks * k2
    core_maps = []
    for m in range(ncores):
        si16 = np.zeros((R, 16, bk * 8), np.int16)
        of = np.full((R, P, bk), 128.0, ml_dtypes.bfloat16)
        gcnt = np.ones((R, 1, nblocks * 2), np.int32)
        for r in range(R):
            sel, dl, blk, sA = per_rm[(r, m)]
            s_r = src[r][sel]
            for b in range(nblocks):
                inb = blk == b
                for hi, (half, koff, kw) in enumerate(
                        ((sA & inb, 0, ka), (~sA & inb, ka, kb))):
                    idx = np.nonzero(half)[0]
                    cnt = len(idx)
                    assert cnt <= kw * P
                    sv = s_r[idx] - (0 if koff == 0 else nh)
                    ov = (dl[idx] - b * P).astype(np.float32)
                    js = np.arange(cnt)
                    of[r, js % P, b * k2 + koff + js // P] = ov
                    # gather slot i = k*128 + p; idx value at [i%16, i//16]
                    flat = np.zeros(kw * P, np.int16)
                    flat[:cnt] = sv.astype(np.int16)
                    gcnt[r, 0, 2 * b + hi] = max(cnt, 1)
                    wrap = flat.reshape(kw * 8, 16).T  # [16, kw*8]
                    si16[r, :, (b * k2 + koff) * 8:(b * k2 + koff + kw) * 8] = wrap
        ofT = np.ascontiguousarray(of.transpose(0, 2, 1))
        cb = np.array([[m * npc]], np.int32)
        core_maps.append(dict(sidx16=np.tile(si16, (1, 8, 1)), offs=of,
                              offsT=ofT, cbase=cb, gcnt=gcnt))

    rep = dict(hT=hT,
               bigW=bigWh.astype(ml_dtypes.bfloat16), bigB=bigBh,
               sdW=sdWh, sdB=sdBh, linW6=linW6, linB=linB)
    return rep, core_maps, nt, nblocks, ka, kb, npc


def _forward(h, dW, db, fW, fb, wW, wb, aW, ab, linW, linb, src, dst,
             ncores=NCORES, trace=False):
    rep, core_maps, nt, nblocks, ka, kb, npc = _host_prep(
        h, dW, db, fW, fb, wW, wb, aW, ab, linW, linb, src, dst, ncores)

    key = (nt, nblocks, ka, kb, ncores)
    if key not in _PROG_CACHE:
        _PROG_CACHE[key] = _build_program(*key)
    nc = _PROG_CACHE[key]

    in_maps = [{**rep, **cm} for cm in core_maps]
    res = run_bass_kernel_spmd(nc, in_maps, list(range(ncores)), trace=trace)
    out = np.concatenate([res.results[m]["out"][:npc] for m in range(ncores)], axis=0)
    return (out, res)


def kernel(**inputs):
    args = [np.asarray(inputs[k]) for k in
            ("h", "dW", "db", "fW", "fb", "wW", "wb", "aW", "ab", "linW", "linb")]
    src = np.asarray(inputs["src"], np.int32)
    dst = np.asarray(inputs["dst"], np.int32)
    out, _ = _forward(*args, src, dst)
    return out


# revision 39
# speedup vs baseline: 1.0468x; 1.0468x over previous
"""Trainium2 Bass kernel: multi-relation GNN message-passing layer (H2FDMultiRelationLayer).

Math folds (exact):
  * sign(tanh(x)) == sign(x); concat([sd,dd,sd-dd]) @ fW == sd@(f1+f3) + dd@(f2-f3)
    with per-node scalars u = h@du + cu, v = h@dv + cv so sign_e = sign(u[src]+v[dst]).
    u, v are relation-independent (dW shared), computed once in fp32.
  * attention logit per head a: alpha[e,a] = leaky_relu(sign_e*p[src,a] + q[dst,a])
    with p = h@(wW@P)+bias, q = h@(wW@Q)+bias+ab  (per-node, per-head scalars).
  * segment softmax without max-subtraction (alpha bounded; exp cannot overflow).

v3 performance structure:
  * All heavy matmuls bf16; fp32 kept only for the [u,v,q*12] node-scalar matmul.
  * Node tables: bigT [R, n_pad, 384] bf16 rows [hw(256)|p(4)|u_hi|u_lo|pad]
    (768B rows — dma_gather needs 256B-aligned rows); sdT [n_pad, 16] fp32.
  * Phase-2 src gathers via InstDMAGatherAnt: ONE instruction per (block,
    relation, node-half) gathers all ~K*128 edge rows (994ns fixed amortized).
    int16 index limit forces the two node-half tables; host splits edge slots
    into the A-half columns [0,KA) and B-half [KA,K2).
  * dst-side per-edge values (v_hi,v_lo,q) expanded on the PE via one-hot mofT
    matmuls from an indirectly-fetched local 128-row slice — no dst gathers.
  * mof/mofT one-hot matrices for ALL K slots built in ONE wide DVE op each;
    pad slots carry dst-offset 128 so their one-hot column is all-zero — pads
    are killed by the scatter matmul itself, no edge mask anywhere.
  * leaky-relu+exp on the Scalar engine; phase-1 bias adds split across
    Vector/GpSimd; biases via partition-broadcast DMA tiles (no bias matmuls).
  * Phase-1 h loads and sdT writes batched 4 tiles per DMA; bigT written once
    per tile for all 3 relations via a transposed DRAM access pattern.

Distribution: nodes partitioned by destination across 8 cores; each core gets
exactly the edges whose dst it owns (host-side selection, dst-sorted, grouped
into 128-row destination blocks). Node tables computed replicated. No collectives.
"""

import math
from contextlib import ExitStack

import numpy as np
import ml_dtypes

import concourse.bass as bass
import concourse.bacc as bacc
import concourse.tile as tile
import concourse.mybir as mybir
from concourse.bass_utils import run_bass_kernel_spmd
from concourse.masks import make_identity

# problem dims (fixed by the nn.Module)
IN = 128          # input feature dim
HF = 64           # per-head hidden
AH = 4            # attention heads
R = 3             # relations
H = AH * HF       # 256
NCORES = 8
P = 128
BW = 260          # per-relation bf16 matmul cols: [hw(256) | p(4)]
TW = 384          # bf16 table row: [hw(256)|p(4)|u_hi|u_lo|pad] — 768B, 256B-aligned
CW = 260          # scatter rhs cols: [hwv(256) | exp(4)]
F32 = mybir.dt.float32
BF16 = mybir.dt.bfloat16
I32 = mybir.dt.int32
I16 = mybir.dt.int16

_PROG_CACHE: dict = {}


def _build_program(nt: int, nblocks: int, ka: int, kb: int, ncores: int):
    """Trace + compile the SPMD device program (same for all cores)."""
    n_pad = nt * P            # padded node-table rows
    awin = min(n_pad, 32768)  # A gather window [0, awin)
    bstart = max(0, n_pad - 32768)  # B gather window [bstart, n_pad)
    k2 = ka + kb              # edge-group slots per block
    bk = nblocks * k2
    npcp = nblocks * P        # padded per-core output rows

    nc = bacc.Bacc("TRN2", target_bir_lowering=False, debug=False, num_devices=ncores)

    hT = nc.dram_tensor("hT", [IN, n_pad], F32, kind="ExternalInput")
    bigW = nc.dram_tensor("bigW", [R, IN, BW], BF16, kind="ExternalInput")
    bigB = nc.dram_tensor("bigB", [R, 1, BW], F32, kind="ExternalInput")
    sdW = nc.dram_tensor("sdW", [IN, 16], F32, kind="ExternalInput")
    sdB = nc.dram_tensor("sdB", [1, 16], F32, kind="ExternalInput")
    linW6 = nc.dram_tensor("linW6", [2 * R, P, H], BF16, kind="ExternalInput")
    linB = nc.dram_tensor("linB", [1, H], F32, kind="ExternalInput")
    sidx16 = nc.dram_tensor("sidx16", [R, P, bk * 8], I16, kind="ExternalInput")
    gcnt = nc.dram_tensor("gcnt", [R, 1, nblocks * 2], I32, kind="ExternalInput")
    offs = nc.dram_tensor("offs", [R, P, bk], BF16, kind="ExternalInput")
    offsT = nc.dram_tensor("offsT", [R, bk, P], BF16, kind="ExternalInput")
    cbase = nc.dram_tensor("cbase", [1, 1], I32, kind="ExternalInput")
    out = nc.dram_tensor("out", [npcp, H], F32, kind="ExternalOutput")

    bigT = nc.dram_tensor("bigT", [R, n_pad, TW], BF16)
    sdT = nc.dram_tensor("sdT", [n_pad, 16], F32)

    with tile.TileContext(nc) as tc:
        with ExitStack() as ctx:
            cpool = ctx.enter_context(tc.tile_pool(name="const", bufs=1))
            iota_i = cpool.tile([P, P], I32)
            nc.gpsimd.iota(iota_i[:], pattern=[[1, P]], base=0, channel_multiplier=0)
            iota_b = cpool.tile([P, P], BF16)
            nc.vector.tensor_copy(iota_b[:], iota_i[:])
            iotac_i = cpool.tile([P, 1], I32)
            nc.gpsimd.iota(iotac_i[:], pattern=[[0, 1]], base=0, channel_multiplier=1)
            iotac_f = cpool.tile([P, 1], F32)
            nc.vector.tensor_copy(iotac_f[:], iotac_i[:])
            ident = cpool.tile([P, P], BF16)
            make_identity(nc, ident[:])

            bw_sb = []
            for r in range(R):
                t = cpool.tile([IN, BW], BF16, tag=f"bw{r}")
                nc.sync.dma_start(t[:], bigW[r, :, :])
                bw_sb.append(t)
            bb_sb = []
            for r in range(R):
                t = cpool.tile([P, BW], F32, tag=f"bb{r}")
                nc.sync.dma_start(t[:], bigB[r, :, :].to_broadcast((P, BW)))
                bb_sb.append(t)
            sdw_sb = cpool.tile([IN, 16], F32)
            nc.sync.dma_start(sdw_sb[:], sdW[:, :])
            sdb_sb = cpool.tile([P, 16], F32)
            nc.sync.dma_start(sdb_sb[:], sdB[:, :].to_broadcast((P, 16)))
            lw_sb = []
            for i in range(2 * R):
                t = cpool.tile([P, H], BF16, tag=f"lw{i}")
                nc.sync.dma_start(t[:], linW6[i, :, :])
                lw_sb.append(t)
            lb_sb = cpool.tile([P, H], F32)
            nc.sync.dma_start(lb_sb[:], linB[:, :].to_broadcast((P, H)))
            cb_sb = cpool.tile([P, 1], I32)
            nc.sync.dma_start(cb_sb[:], cbase[:, :].to_broadcast((P, 1)))
            cb_f = cpool.tile([P, 1], F32)
            nc.vector.tensor_copy(cb_f[:], cb_sb[:])
            si_sb, of_sb, gc_sb = [], [], []
            for r in range(R):
                a = cpool.tile([P, bk * 8], I16, tag=f"si{r}")
                nc.sync.dma_start(a[:], sidx16[r, :, :])
                si_sb.append(a)
                a = cpool.tile([P, bk], BF16, tag=f"of{r}")
                nc.sync.dma_start(a[:], offs[r, :, :])
                of_sb.append(a)
                a = cpool.tile([1, nblocks * 2], I32, tag=f"gc{r}")
                nc.sync.dma_start(a[:], gcnt[r, :, :])
                gc_sb.append(a)

            # ---------------- phase 1: node tables ----------------
            GH = 4  # h-load / sdT-write batching
            with tc.tile_pool(name="p1h", bufs=2) as hp, \
                 tc.tile_pool(name="p1ps", bufs=4, space="PSUM") as pp, \
                 tc.tile_pool(name="p1sd", bufs=2, space="PSUM") as sp, \
                 tc.tile_pool(name="p1o", bufs=3) as op, \
                 tc.tile_pool(name="p1s", bufs=2) as sdp:
                for tg in range(0, nt, GH):
                    g = min(GH, nt - tg)
                    ht4 = hp.tile([IN, GH * P], F32, tag="ht4")
                    nc.sync.dma_start(ht4[:, 0:g * P], hT[:, tg * P:(tg + g) * P])
                    htb4 = hp.tile([IN, GH * P], BF16, tag="htb4")
                    nc.gpsimd.tensor_copy(htb4[:, 0:g * P], ht4[:, 0:g * P])
                    sd4 = sdp.tile([P, GH * 16], F32)
                    for j in range(g):
                        t = tg + j
                        ht = ht4[:, j * P:(j + 1) * P]
                        htb = htb4[:, j * P:(j + 1) * P]

                        # fp32 node scalars [u, v, q*12]
                        ps_sd = sp.tile([P, 16], F32)
                        nc.tensor.matmul(ps_sd[:], lhsT=ht, rhs=sdw_sb[:],
                                         start=True, stop=True)
                        sd = sd4[:, j * 16:(j + 1) * 16]
                        nc.vector.tensor_add(sd, ps_sd[:], sdb_sb[:])

                        # u -> bf16 hi/lo (shared across relations)
                        uhl = op.tile([P, 2], BF16, tag="uhl")
                        nc.gpsimd.tensor_copy(uhl[:, 0:1], sd[:, 0:1])
                        uhf = op.tile([P, 1], F32, tag="uhf")
                        nc.gpsimd.tensor_copy(uhf[:], uhl[:, 0:1])
                        nc.gpsimd.tensor_sub(uhl[:, 1:2], sd[:, 0:1], uhf[:])

                        hw3 = op.tile([P, R * TW], BF16, tag="hw3")
                        hw3v = hw3[:].rearrange("p (r c) -> p r c", r=R)
                        for r in range(R):
                            ps = pp.tile([P, BW], F32)
                            nc.tensor.matmul(ps[:], lhsT=htb, rhs=bw_sb[r][:],
                                             start=True, stop=True)
                            nc.vector.tensor_add(hw3[:, r * TW:r * TW + BW],
                                                 ps[:], bb_sb[r][:])
                        nc.gpsimd.tensor_copy(
                            hw3v[:, :, BW:BW + 2],
                            uhl[:].rearrange("p (o c) -> p o c", o=1)
                            .to_broadcast((P, R, 2)))
                        nc.sync.dma_start(
                            bigT[:, t * P:(t + 1) * P, :].rearrange(
                                "r n c -> n r c"),
                            hw3v)
                    nc.sync.dma_start(
                        sdT[tg * P:(tg + g) * P, :].rearrange(
                            "(j n) c -> n j c", j=g),
                        sd4[:, 0:g * 16].rearrange("p (j c) -> p j c", j=g))

            # ---------------- phase 2: edge aggregation ----------------
            with tc.tile_pool(name="gth", bufs=4) as gp, \
                 tc.tile_pool(name="cc", bufs=3) as cp, \
                 tc.tile_pool(name="mo", bufs=3) as mp, \
                 tc.tile_pool(name="sml", bufs=4) as smlp, \
                 tc.tile_pool(name="dv", bufs=3) as dvp, \
                 tc.tile_pool(name="pex", bufs=2, space="PSUM") as pexp, \
                 tc.tile_pool(name="pblk", bufs=2, space="PSUM") as pblkp, \
                 tc.tile_pool(name="pout", bufs=2, space="PSUM") as poutp, \
                 tc.tile_pool(name="ptp", bufs=2, space="PSUM") as ptpp, \
                 tc.tile_pool(name="nrm", bufs=4) as nrmp, \
                 tc.tile_pool(name="obp", bufs=3) as obp:
                # prime the gather buffers: slots past the per-block dynamic
                # gather count are never written, so scrub any NaN bit
                # patterns that fresh SBUF might hold (downstream math must
                # stay finite; pad slots contribute 0 via all-zero one-hots).
                for _ in range(4):
                    t = gp.tile([P, k2 * TW], BF16, tag="gth")
                    nc.vector.memset(t[:], 0.0)
                for b in range(nblocks):
                    pso = poutp.tile([P, H], F32)
                    # local dst-block node scalars (core-dependent rows)
                    lidx_f = dvp.tile([P, 1], F32, tag="lidxf")
                    nc.vector.tensor_scalar(
                        out=lidx_f[:], in0=iotac_f[:],
                        scalar1=cb_f[:], scalar2=float(b * P),
                        op0=mybir.AluOpType.add, op1=mybir.AluOpType.add)
                    lidx = dvp.tile([P, 1], I32, tag="lidx")
                    nc.vector.tensor_copy(lidx[:], lidx_f[:])
                    sdl = dvp.tile([P, 16], F32, tag="sdl")
                    nc.gpsimd.indirect_dma_start(
                        out=sdl[:], out_offset=None,
                        in_=sdT[:, :],
                        in_offset=bass.IndirectOffsetOnAxis(
                            ap=lidx[:, 0:1], axis=0),
                        element_offset=0)
                    # v -> bf16 hi/lo (shared across relations)
                    vhl = dvp.tile([P, 2], BF16, tag="vhl")
                    nc.vector.tensor_copy(vhl[:, 0:1], sdl[:, 1:2])
                    vhf = dvp.tile([P, 1], F32, tag="vhf")
                    nc.vector.tensor_copy(vhf[:], vhl[:, 0:1])
                    nc.vector.tensor_sub(vhl[:, 1:2], sdl[:, 1:2], vhf[:])

                    for r in range(R):
                        csl = slice(b * k2, (b + 1) * k2)
                        dstv = dvp.tile([P, 8], BF16, tag="dstv")
                        nc.vector.tensor_copy(dstv[:, 0:2], vhl[:])
                        nc.vector.tensor_copy(dstv[:, 2:6],
                                              sdl[:, 2 + 4 * r:6 + 4 * r])

                        # src gathers: one InstDMAGatherAnt per node-half
                        gth = gp.tile([P, k2 * TW], BF16, tag="gth")
                        nc.gpsimd.dma_gather(
                            out_ap=gth[:, 0:ka * TW].rearrange(
                                "p (k e) -> p k e", k=ka),
                            in_ap=bigT[r, 0:awin, :],
                            idxs_ap=si_sb[r][:, b * k2 * 8:(b * k2 + ka) * 8],
                            num_idxs=ka * P, num_idxs_reg=ka * P,
                            elem_size=TW)
                        nc.gpsimd.dma_gather(
                            out_ap=gth[:, ka * TW:].rearrange(
                                "p (k e) -> p k e", k=kb),
                            in_ap=bigT[r, bstart:n_pad, :],
                            idxs_ap=si_sb[r][:, (b * k2 + ka) * 8:
                                             (b + 1) * k2 * 8],
                            num_idxs=kb * P, num_idxs_reg=kb * P,
                            elem_size=TW)

                        # one-hot matrices for ALL k2 slots, one wide op each;
                        # pad slots have of==128 -> all-zero one-hot column.
                        otbs = mp.tile([P, k2 * P], BF16, tag="otbs")
                        nc.sync.dma_start(
                            otbs[:].rearrange("p (k j) -> p k j", k=k2),
                            offsT[r:r + 1, b * k2:(b + 1) * k2, :]
                            .to_broadcast((P, k2, P)))
                        mofT = mp.tile([P, k2 * P], BF16, tag="mofT")
                        nc.vector.tensor_scalar(
                            out=mofT[:], in0=otbs[:],
                            scalar1=iotac_f[:], scalar2=None,
                            op0=mybir.AluOpType.is_equal)
                        mof = mp.tile([P, k2 * P], BF16, tag="mof")
                        nc.vector.tensor_tensor(
                            out=mof[:].rearrange("p (k j) -> p k j", k=k2),
                            in0=iota_b[:].rearrange("p (o j) -> p o j", o=1)
                            .to_broadcast((P, k2, P)),
                            in1=of_sb[r][:, csl].rearrange("p (k o) -> p k o", o=1)
                            .to_broadcast((P, k2, P)),
                            op=mybir.AluOpType.is_equal)

                        # dst-side expand: pex[p, k*8+c] = dstv[of[p,k], c]
                        pex = pexp.tile([P, k2 * 8], F32)
                        for k in range(k2):
                            nc.tensor.matmul(pex[:, k * 8:(k + 1) * 8],
                                             lhsT=mofT[:, k * P:(k + 1) * P],
                                             rhs=dstv[:],
                                             start=True, stop=True)

                        gv = gth[:].rearrange("p (k e) -> p k e", k=k2)
                        ev = pex[:].rearrange("p (k e) -> p k e", k=k2)
                        # score = (u_hi+u_lo) + (v_hi+v_lo); sign
                        scu = smlp.tile([P, k2], F32, tag="scu")
                        scu3 = scu[:].rearrange("p (k o) -> p k o", o=1)
                        nc.vector.tensor_add(scu3, gv[:, :, BW:BW + 1],
                                             gv[:, :, BW + 1:BW + 2])
                        sc = smlp.tile([P, k2], F32, tag="sc")
                        sc3 = sc[:].rearrange("p (k o) -> p k o", o=1)
                        nc.vector.tensor_add(sc3, scu3, ev[:, :, 0:1])
                        nc.vector.tensor_add(sc3, sc3, ev[:, :, 1:2])
                        sgn = smlp.tile([P, k2], BF16, tag="sgn")
                        nc.scalar.sign(sgn[:], sc[:])
                        sgnb = sgn[:].rearrange("p (k o) -> p k o", o=1) \
                            .to_broadcast([P, k2, AH])

                        # alpha = sign*p + q ; leaky relu (ACT); exp (ACT)
                        spq = smlp.tile([P, k2 * AH], F32, tag="spq")
                        spq3 = spq[:].rearrange("p (k a) -> p k a", k=k2)
                        nc.vector.tensor_tensor(out=spq3, in0=gv[:, :, 256:260],
                                                in1=sgnb, op=mybir.AluOpType.mult)
                        nc.vector.tensor_tensor(out=spq3, in0=spq3,
                                                in1=ev[:, :, 2:6],
                                                op=mybir.AluOpType.add)
                        zl = smlp.tile([P, k2 * AH], F32, tag="zl")
                        nc.vector.tensor_scalar_mul(zl[:], spq[:], 0.01)
                        nc.vector.tensor_max(zl[:], zl[:], spq[:])
                        C = cp.tile([P, k2 * CW], BF16)
                        Cv = C[:].rearrange("p (k c) -> p k c", c=CW)
                        exv = Cv[:, :, 256:260]
                        nc.scalar.activation(exv, zl[:].rearrange(
                            "p (k a) -> p k a", k=k2),
                            mybir.ActivationFunctionType.Exp)
                        co = smlp.tile([P, k2 * AH], BF16, tag="co")
                        co3 = co[:].rearrange("p (k a) -> p k a", k=k2)
                        nc.vector.tensor_tensor(out=co3, in0=exv, in1=sgnb,
                                                op=mybir.AluOpType.mult)
                        hwv = Cv[:, :, 0:256].rearrange(
                            "p k (a f) -> p k a f", a=AH)
                        cob = co[:].rearrange("p (k a) -> p k a", k=k2) \
                            .to_broadcast([P, k2, AH, HF])
                        nc.vector.tensor_tensor(
                            out=hwv,
                            in0=gv[:, :, 0:256].rearrange(
                                "p k (a f) -> p k a f", a=AH),
                            in1=cob, op=mybir.AluOpType.mult)

                        # scatter: psb[j, :] = sum_p mof[p,j] * C[p, :]
                        psb = pblkp.tile([P, CW], F32)
                        for k in range(k2):
                            nc.tensor.matmul(psb[:],
                                             lhsT=mof[:, k * P:(k + 1) * P],
                                             rhs=C[:, k * CW:(k + 1) * CW],
                                             start=(k == 0), stop=(k == k2 - 1))

                        den = smlp.tile([P, AH], F32, tag="den")
                        nc.vector.tensor_scalar_max(den[:], psb[:, 256:260], 1e-30)
                        rec = smlp.tile([P, AH], F32, tag="rec")
                        nc.vector.reciprocal(rec[:], den[:])
                        nrm = nrmp.tile([P, H], BF16, tag="nrm")
                        nrm3 = nrm[:].rearrange("p (a f) -> p a f", a=AH)
                        nc.vector.tensor_tensor(
                            out=nrm3,
                            in0=psb[:, 0:256].rearrange("p (a f) -> p a f", a=AH),
                            in1=rec[:].to_broadcast([P, AH, HF]),
                            op=mybir.AluOpType.mult)
                        for ch in range(2):
                            pt = ptpp.tile([P, P], BF16)
                            nc.tensor.transpose(pt[:], nrm[:, ch * P:(ch + 1) * P],
                                                ident[:])
                            ntc = nrmp.tile([P, P], BF16, tag="ntc")
                            nc.vector.tensor_copy(ntc[:], pt[:])
                            nc.tensor.matmul(pso[:], lhsT=ntc[:],
                                             rhs=lw_sb[2 * r + ch][:],
                                             start=(r == 0 and ch == 0),
                                             stop=(r == R - 1 and ch == 1))
                    ob = obp.tile([P, H], F32)
                    nc.vector.tensor_add(ob[:], pso[:], lb_sb[:])
                    nc.sync.dma_start(out[b * P:(b + 1) * P, :], ob[:])

    nc.compile()
    return nc


def _host_prep(h, dW, db, fW, fb, wW, wb, aW, ab, linW, linb, src, dst, ncores):
    """Fold weights + partition/sort edges by destination owner."""
    n = h.shape[0]
    npc = n // ncores
    assert npc * ncores == n
    nblocks = math.ceil(npc / P)
    nt = math.ceil(n / P)
    n_pad = nt * P
    hT = np.zeros((IN, n_pad), np.float32)
    hT[:, :n] = np.ascontiguousarray(h.T)

    fW1, fW2, fW3 = fW[0:H, 0], fW[H:2 * H, 0], fW[2 * H:3 * H, 0]
    du = dW @ (fW1 + fW3)
    dv = dW @ (fW2 - fW3)
    cu = float(db @ (fW1 + fW3) + fb[0])
    cv = float(db @ (fW2 - fW3))

    sdWh = np.zeros((IN, 16), np.float32)
    sdBh = np.zeros((1, 16), np.float32)
    sdWh[:, 0] = du
    sdWh[:, 1] = dv
    sdBh[0, 0] = cu
    sdBh[0, 1] = cv
    bigWh = np.zeros((R, IN, BW), np.float32)
    bigBh = np.zeros((R, 1, BW), np.float32)
    for r in range(R):
        Pm = np.zeros((H, AH), np.float32)
        Qm = np.zeros((H, AH), np.float32)
        for a in range(AH):
            Pm[a * HF:(a + 1) * HF, a] = aW[r, :HF, 0]
            Qm[a * HF:(a + 1) * HF, a] = aW[r, HF:, 0]
        bigWh[r, :, 0:H] = wW[r]
        bigWh[r, :, 256:260] = wW[r] @ Pm
        bigBh[r, 0, 0:H] = wb[r]
        bigBh[r, 0, 256:260] = wb[r] @ Pm
        sdWh[:, 2 + 4 * r:6 + 4 * r] = wW[r] @ Qm
        sdBh[0, 2 + 4 * r:6 + 4 * r] = wb[r] @ Qm + ab[r, 0]

    linW6 = np.ascontiguousarray(
        linW.reshape(2 * R, P, H)).astype(ml_dtypes.bfloat16)
    linB = linb.reshape(1, H).astype(np.float32)

    # --- edge partition: per (relation, core) sort by dst; assign each edge to
    # the A gather window [0, awin) or B window [bstart, n_pad). The windows
    # overlap, so edges in the overlap are flexible — balance per block to
    # minimize slot count. ---
    awin = min(n_pad, 32768)
    bstart = max(0, n_pad - 32768)
    owner = [dst[r] // npc for r in range(R)]
    per_rm = {}
    stats = []
    for r in range(R):
        for m in range(ncores):
            sel = np.nonzero(owner[r] == m)[0]
            dl = dst[r][sel] - m * npc
            order = np.argsort(dl, kind="stable")
            sel = sel[order]
            dl = dl[order]
            blk = dl // P
            s_r = src[r][sel]
            onlyA = s_r < bstart           # below B window
            onlyB = s_r >= awin            # above A window
            cAf = np.bincount(blk[onlyA], minlength=nblocks)
            cBf = np.bincount(blk[onlyB], minlength=nblocks)
            cT = np.bincount(blk, minlength=nblocks)
            stats.append((cAf, cBf, cT))
            per_rm[(r, m)] = (sel, dl, blk, onlyA, onlyB)

    # minimal (ka, kb): smallest k2 such that every block fits
    maxAf = max(int(c[0].max()) for c in stats)
    maxBf = max(int(c[1].max()) for c in stats)
    maxT = max(int(c[2].max()) for c in stats)
    best = None
    for k2try in range(int(math.ceil(maxT / P)), 64):
        for katry in range(1, k2try):
            kbtry = k2try - katry
            if maxAf <= katry * P and maxBf <= kbtry * P:
                best = (katry, kbtry)
                break
        if best:
            break
    assert best is not None
    ka, kb = best

    k2 = ka + kb
    bk = nblocks * k2
    core_maps = []
    for m in range(ncores):
        si16 = np.zeros((R, 16, bk * 8), np.int16)
        of = np.full((R, P, bk), 128.0, ml_dtypes.bfloat16)
        gcnt = np.ones((R, 1, nblocks * 2), np.int32)
        for r in range(R):
            sel, dl, blk, onlyA, onlyB = per_rm[(r, m)]
            s_r = src[r][sel]
            for b in range(nblocks):
                inb = blk == b
                # balance flexible (overlap-window) edges across A/B
                fxA = np.nonzero(onlyA & inb)[0]
                fxB = np.nonzero(onlyB & inb)[0]
                fl = np.nonzero(inb & ~onlyA & ~onlyB)[0]
                f = len(fl)
                xmin = max(0, len(fxB) + f - kb * P)
                xmax = min(f, ka * P - len(fxA))
                x = max(xmin, min(xmax, (f + len(fxB) - len(fxA)) // 2))
                assert xmin <= xmax
                halves = (np.concatenate([fxA, fl[:x]]),
                          np.concatenate([fxB, fl[x:]]))
                for hi, (idx, koff, kw, base) in enumerate(
                        ((halves[0], 0, ka, 0), (halves[1], ka, kb, bstart))):
                    cnt = len(idx)
                    assert cnt <= kw * P
                    sv = s_r[idx] - base
                    ov = (dl[idx] - b * P).astype(np.float32)
                    js = np.arange(cnt)
                    of[r, js % P, b * k2 + koff + js // P] = ov
                    # gather slot i = k*128 + p; idx value at [i%16, i//16]
                    flat = np.zeros(kw * P, np.int16)
                    flat[:cnt] = sv.astype(np.int16)
                    gcnt[r, 0, 2 * b + hi] = max(cnt, 1)
                    wrap = flat.reshape(kw * 8, 16).T  # [16, kw*8]
                    si16[r, :, (b * k2 + koff) * 8:(b * k2 + koff + kw) * 8] = wrap
        ofT = np.ascontiguousarray(of.transpose(0, 2, 1))
        cb = np.array([[m * npc]], np.int32)
        core_maps.append(dict(sidx16=np.tile(si16, (1, 8, 1)), offs=of,
                              offsT=ofT, cbase=cb, gcnt=gcnt))

    rep = dict(hT=hT,
               bigW=bigWh.astype(ml_dtypes.bfloat16), bigB=bigBh,
               sdW=sdWh, sdB=sdBh, linW6=linW6, linB=linB)
    return rep, core_maps, nt, nblocks, ka, kb, npc


def _forward(h, dW, db, fW, fb, wW, wb, aW, ab, linW, linb, src, dst,
             ncores=NCORES, trace=False):
    rep, core_maps, nt, nblocks, ka, kb, npc = _host_prep(
        h, dW, db, fW, fb, wW, wb, aW, ab, linW, linb, src, dst, ncores)

    key = (nt, nblocks, ka, kb, ncores)
    if key not in _PROG_CACHE:
        _PROG_CACHE[key] = _build_program(*key)
    nc = _PROG_CACHE[key]

    in_maps = [{**rep, **cm} for cm in core_maps]
    res = run_bass_kernel_spmd(nc, in_maps, list(range(ncores)), trace=trace)
    out = np.concatenate([res.results[m]["out"][:npc] for m in range(ncores)], axis=0)
    return (out, res)


def kernel(**inputs):
    args = [np.asarray(inputs[k]) for k in
            ("h", "dW", "db", "fW", "fb", "wW", "wb", "aW", "ab", "linW", "linb")]
    src = np.asarray(inputs["src"], np.int32)
    dst = np.asarray(inputs["dst"], np.int32)
    out, _ = _forward(*args, src, dst)
    return out
